# revision 2
# baseline (speedup 1.0000x reference)
"""Causal MHA with RoPE on 8 trn2 cores — v3.

Sharding: core = (batch b, head-group g). b = core//2, g = core%2.
Each core computes 8 heads of one batch and a partial output projection;
host sums the two partials per batch.

vs baseline:
- PV matmuls flipped to [q=128, 65] layout (K=128 contraction, ~2x less PE).
- Softmax denominator comes out as a column (ones col in V); normalize is a
  DVE reciprocal + broadcast multiply (no Ln/Exp/broadcast-matmuls).
- All transposes moved to the DMA xbar (dma_start_transpose), freeing PE+PSUM.
- Rope reads the projection PSUM directly (no ACT staging copy).
- Causal-masked score columns are skipped in scores/exp/PV where fully masked.
- Score groups hold one k-chunk for BOTH heads of a pair: one exp serves two
  heads; PV ping-pongs between the pair's two accumulators.
- Output projection of block p is spread through attention of block p+1.
"""

import numpy as np
import ml_dtypes

import concourse.bass as bass
from concourse import bacc
import concourse.mybir as mybir
import concourse.tile as tile
from concourse.bass_utils import run_bass_kernel_spmd

F32 = mybir.dt.float32
BF16 = mybir.dt.bfloat16
AF = mybir.ActivationFunctionType

P = 128
S = 2048          # sequence length
DM = 1024         # model dim
DH = 512          # per-core heads dim (8 heads x 64)
DK = 64
NHL = 8           # local heads
KT = DM // P      # 8 k-tiles for projections
SC = S // P       # 16 s-chunks
QB = S // 512     # 4 q-blocks
THETA = 10000.0


def build_program():
    nc = bacc.Bacc("TRN2", target_bir_lowering=False, debug=False)
    xt_d = nc.dram_tensor("xt", [P, KT, S], BF16, kind="ExternalInput").ap()
    wqt_d = nc.dram_tensor("wqt", [P, KT, DH], BF16, kind="ExternalInput").ap()
    wkt_d = nc.dram_tensor("wkt", [P, KT, DH], BF16, kind="ExternalInput").ap()
    wvt_d = nc.dram_tensor("wvt", [P, KT, DH], BF16, kind="ExternalInput").ap()
    wot_d = nc.dram_tensor("wot", [P, 4, DM], BF16, kind="ExternalInput").ap()
    ct_d = nc.dram_tensor("ct", [P, SC, 32], BF16, kind="ExternalInput").ap()
    st_d = nc.dram_tensor("st", [P, SC, 64], BF16, kind="ExternalInput").ap()
    mask_d = nc.dram_tensor("mask", [P, 512], BF16, kind="ExternalInput").ap()
    yt_d = nc.dram_tensor("yt", [DM, S], F32, kind="ExternalOutput").ap()

    with tile.TileContext(nc) as tc:
        with (
            tc.tile_pool(name="consts", bufs=1) as consts,
            tc.tile_pool(name="rope", bufs=3) as rope_pool,
            tc.tile_pool(name="ptp", bufs=5) as ptp,
            tc.tile_pool(name="opn", bufs=3) as opn_pool,
            tc.tile_pool(name="otn", bufs=9) as otn_pool,
            tc.tile_pool(name="rcp", bufs=4) as rcp_pool,
            tc.tile_pool(name="ysb", bufs=3) as ysb_pool,
            tc.tile_pool(name="pssc", bufs=2, space="PSUM") as ps_sc,
            tc.tile_pool(name="pso", bufs=1, space="PSUM") as ps_o,
            tc.tile_pool(name="psw", bufs=2, space="PSUM") as ps_w,
        ):
            # ---- constants / persistent state ----
            xt = consts.tile([P, KT, S], BF16)
            wqt = consts.tile([P, KT, DH], BF16)
            wkt = consts.tile([P, KT, DH], BF16)
            wvt = consts.tile([P, KT, DH], BF16)
            wot = consts.tile([P, 4, DM], BF16)
            ctbl = consts.tile([P, SC, 32], BF16)
            stbl = consts.tile([P, SC, 64], BF16)
            mask = consts.tile([P, 512], BF16)

            # interleave x/weight loads so the first proj matmuls start early
            for kk in range(0, KT, 2):
                nc.sync.dma_start(out=xt[:, kk:kk + 2, 0:512],
                                  in_=xt_d[:, kk:kk + 2, 0:512])
                nc.sync.dma_start(out=wqt[:, kk:kk + 2, :],
                                  in_=wqt_d[:, kk:kk + 2, :])
            nc.sync.dma_start(out=ctbl, in_=ct_d)
            nc.sync.dma_start(out=stbl, in_=st_d)
            nc.sync.dma_start(out=wkt, in_=wkt_d)
            nc.sync.dma_start(out=wvt, in_=wvt_d)
            nc.sync.dma_start(out=mask, in_=mask_d)
            for sb in range(1, 4):
                nc.sync.dma_start(out=xt[:, :, sb * 512:(sb + 1) * 512],
                                  in_=xt_d[:, :, sb * 512:(sb + 1) * 512])
            nc.sync.dma_start(out=wot, in_=wot_d)

            qt_store = consts.tile([P, 4, S], BF16)
            kt_store = consts.tile([P, 4, S], BF16)
            v_aug = consts.tile([P, SC, NHL * 65], BF16)
            # only the ones-columns (col 64 of each head slot) need init
            nc.vector.memset(
                v_aug.rearrange("p s (h c) -> p s h c", h=NHL)[:, :, :, 64:65],
                1.0)

            # ---- projections + rope, per s-chunk; transposes on DMA xbar ----
            def rope_to(pj, sc, dst_store):
                # pj: psum [128 s, 512 dh] projection result (pre-rope), f32.
                # out = pj*cos + swap_pairs(pj)*sin_signed, written bf16 SBUF.
                pjv = pj.rearrange("p (h two k) -> p h two k", two=2, k=32)
                # swapped view of pj: "two" dim reversed (negative stride)
                swap = bass.AP(
                    tensor=pjv.tensor, offset=pjv.offset + 32,
                    ap=[list(pjv.ap[0]), list(pjv.ap[1]), [-32, 2], [1, 32]])
                t1 = rope_pool.tile([P, DH], BF16, tag="t1")
                nc.vector.tensor_mul(
                    t1.rearrange("p (h two k) -> p h two k", two=2, k=32),
                    swap,
                    stbl[:, sc, :].rearrange("p (two k) -> p two k", two=2)
                        .unsqueeze(1).broadcast_to([P, NHL, 2, 32]),
                )
                t2 = rope_pool.tile([P, DH], BF16, tag="t2")
                nc.vector.tensor_mul(
                    t2.rearrange("p (hh k) -> p hh k", k=32),
                    pj.rearrange("p (hh k) -> p hh k", k=32),
                    ctbl[:, sc, :].unsqueeze(1).broadcast_to([P, 2 * NHL, 32]),
                )
                qn = rope_pool.tile([P, DH], BF16, tag="qn")
                nc.vector.tensor_add(qn, t1, t2)
                # [128 s, 512 dh] -> [128 dh', 4 pair, 128 s] via DMA xbar
                nc.sync.dma_start_transpose(
                    out=dst_store[:, :, sc * P:(sc + 1) * P], in_=qn)

            def proj_chunk(sc):
                for which in range(3):
                    pj = ps_w.tile([P, DH], F32, tag="w")
                    wt = (wqt, wkt, wvt)[which]
                    for kt in range(KT):
                        nc.tensor.matmul(
                            pj, xt[:, kt, sc * P:(sc + 1) * P], wt[:, kt, :],
                            start=(kt == 0), stop=(kt == KT - 1))
                    if which == 0:
                        rope_to(pj, sc, qt_store)
                    elif which == 1:
                        rope_to(pj, sc, kt_store)
                    else:
                        nc.vector.tensor_copy(
                            v_aug[:, sc, :]
                                .rearrange("p (h c) -> p h c", c=65)[:, :, 0:64],
                            pj.rearrange("p (h c) -> p h c", c=64))

            # ---- attention for one q-block, one head pair ----
            def attn_pair(qb, pair):
                nchunks = 4 * (qb + 1)
                oA = ps_o.tile([P, 4, 65], F32, tag="oA")
                oB = ps_o.tile([P, 4, 65], F32, tag="oB")
                for c in range(nchunks):
                    cr = c - 4 * qb  # >=0 on diagonal chunks
                    lo = 128 * cr if cr > 0 else 0
                    sps = ps_sc.tile([P, 2, 512], F32, tag="sc")
                    pt = ptp.tile([P, 2, 512], BF16, tag="pt")
                    for a in range(2):
                        nc.tensor.matmul(
                            sps[:, a, lo:512],
                            kt_store[64 * a:64 * a + 64, pair,
                                     c * P:(c + 1) * P],
                            qt_store[64 * a:64 * a + 64, pair,
                                     qb * 512 + lo:(qb + 1) * 512],
                            start=True, stop=True)
                    nc.scalar.activation(out=pt[:, :, lo:512],
                                         in_=sps[:, :, lo:512], func=AF.Exp)
                    if cr >= 0:
                        nc.vector.tensor_mul(
                            pt[:, :, lo:512], pt[:, :, lo:512],
                            mask[:, 0:512 - lo].unsqueeze(1)
                                .broadcast_to([P, 2, 512 - lo]))
                    for a, o in ((0, oA), (1, oB)):
                        h = 2 * pair + a
                        # one accumulation group per PSUM bank: start on the
                        # first matmul touching the o bank, stop on the last
                        for qc in range(4):
                            if cr > qc:
                                continue  # fully masked block
                            nc.tensor.matmul(
                                o[:, qc, :],
                                pt[:, a, qc * P:(qc + 1) * P],
                                v_aug[:, c, h * 65:(h + 1) * 65],
                                start=(c == 0 and qc == 0),
                                stop=(c == nchunks - 1 and qc == 3))
                # normalize: col 64 holds the softmax denominator
                oPn = opn_pool.tile([P, 4, P], BF16, tag="opn")
                for a, o in ((0, oA), (1, oB)):
                    r = rcp_pool.tile([P, 4], F32, tag="r")
                    nc.vector.reciprocal(r, o[:, :, 64])
                    nc.vector.tensor_mul(
                        oPn[:, :, 64 * a:64 * a + 64],
                        o[:, :, 0:64],
                        r.unsqueeze(2).broadcast_to([P, 4, 64]))
                otn = otn_pool.tile([P, 4, P], BF16, tag="otn")
                nc.sync.dma_start_transpose(out=otn, in_=oPn)
                return otn

            def outproj(qb, otn_tiles, ecs):
                for ec in ecs:
                    yps = ps_w.tile([P, 512], F32, tag="w")
                    for p_i in range(4):
                        nc.tensor.matmul(
                            yps, wot[:, p_i, ec * P:(ec + 1) * P],
                            otn_tiles[p_i],
                            start=(p_i == 0), stop=(p_i == 3))
                    ysb = ysb_pool.tile([P, 512], F32, tag="ysb")
                    nc.vector.tensor_copy(ysb, yps)
                    nc.sync.dma_start(
                        out=yt_d[ec * P:(ec + 1) * P, qb * 512:(qb + 1) * 512],
                        in_=ysb)

            # ---- interleaved emission ----
            # block 0 projections up-front; then per block: attention pairs
            # interleaved with next-block projections and prev-block outproj.
            for sc in range(4):
                proj_chunk(sc)
            otn_prev = None
            for p_blk in range(QB):
                otn_cur = []
                for pair in range(4):
                    if p_blk < QB - 1:
                        proj_chunk(4 * (p_blk + 1) + pair)
                    otn_cur.append(attn_pair(p_blk, pair))
                    if otn_prev is not None:
                        outproj(p_blk - 1, otn_prev, (2 * pair, 2 * pair + 1))
                otn_prev = otn_cur
            outproj(QB - 1, otn_prev, range(8))

    nc.compile()
    return nc


_NC = None


def _get_program():
    global _NC
    if _NC is None:
        _NC = build_program()
    return _NC


def _prep_inputs(x, token_positions, Wq, Wk, Wv, Wo):
    B = x.shape[0]
    bf = ml_dtypes.bfloat16
    # rope tables from token_positions
    pos = np.asarray(token_positions, dtype=np.float64)
    k = np.arange(1, 33, dtype=np.float64)
    denom = np.power(THETA, 2.0 * (k - 1.0) / 64.0)
    ang = pos[:, None] / denom[None, :]              # [S, 32]
    cos_t = np.cos(ang).astype(np.float32)
    sin_t = np.sin(ang).astype(np.float32)
    ct = cos_t.reshape(SC, P, 32).transpose(1, 0, 2).astype(bf)    # [128, 16, 32]
    st = np.concatenate([-sin_t, sin_t], axis=1)                   # [S, 64]
    st = st.reshape(SC, P, 64).transpose(1, 0, 2).astype(bf)       # [128, 16, 64]

    # deinterleave permutation within each head (evens then odds)
    permh = np.concatenate([np.arange(0, 64, 2), np.arange(1, 64, 2)])
    perm = (np.arange(16)[:, None] * 64 + permh[None, :]).reshape(-1)  # [1024]

    # causal mask for a diagonal chunk: valid iff q_rel >= k_partition
    pidx = np.arange(P)[:, None]
    qidx = np.arange(512)[None, :]
    mask = (qidx >= pidx).astype(bf)                               # [128, 512]

    scale = 1.0 / np.sqrt(np.float32(DK))
    Wq_s = np.asarray(Wq, np.float32) * scale
    Wk_s = np.asarray(Wk, np.float32)
    Wv_s = np.asarray(Wv, np.float32)
    Wo_s = np.asarray(Wo, np.float32)

    in_maps = []
    for b in range(B):
        xT = np.ascontiguousarray(np.asarray(x[b], np.float32).T).astype(bf)   # [1024, 2048]
        xt_h = xT.reshape(KT, P, S).transpose(1, 0, 2)
        for g in range(2):
            rows = slice(g * DH, (g + 1) * DH)
            wq_g = Wq_s[perm[rows], :]        # [512, 1024] permuted rows of this group
            wk_g = Wk_s[perm[rows], :]
            wv_g = Wv_s[rows, :]              # natural order
            wqt_h = wq_g.T.reshape(KT, P, DH).transpose(1, 0, 2).astype(bf)
            wkt_h = wk_g.T.reshape(KT, P, DH).transpose(1, 0, 2).astype(bf)
            wvt_h = wv_g.T.reshape(KT, P, DH).transpose(1, 0, 2).astype(bf)
            wo_g = Wo_s[:, g * DH:(g + 1) * DH]          # [1024, 512]
            wot_h = wo_g.T.reshape(4, P, DM).transpose(1, 0, 2).astype(bf)
            in_maps.append({
                "xt": np.ascontiguousarray(xt_h),
                "wqt": np.ascontiguousarray(wqt_h),
                "wkt": np.ascontiguousarray(wkt_h),
                "wvt": np.ascontiguousarray(wvt_h),
                "wot": np.ascontiguousarray(wot_h),
                "ct": np.ascontiguousarray(ct),
                "st": np.ascontiguousarray(st),
                "mask": np.ascontiguousarray(mask),
            })
    return in_maps


def kernel(x, token_positions, Wq, Wk, Wv, Wo, _trace=False):
    nc = _get_program()
    in_maps = _prep_inputs(x, token_positions, Wq, Wk, Wv, Wo)
    res = run_bass_kernel_spmd(nc, in_maps, list(range(8)), trace=_trace)
    B = x.shape[0]
    out = np.zeros((B, S, DM), np.float32)
    for b in range(B):
        for g in range(2):
            out[b] += res.results[2 * b + g]["yt"].T
    if _trace:
        return out, res
    return out
